# revision 2
# baseline (speedup 1.0000x reference)
"""Trainium2 Bass kernel for nn_Loop_Projection (batched per-prototype GEMM).

Computes out[b, e, p] = sum_d x[b, d, p] * W[p, d, e] + b[p, e] with
x: [256, 512, 128] f32, W: [128, 512, 128] f32, b: [128, 128] f32.

Sharding: prototype axis P=128 split across 8 NeuronCores (16 protos each).
The problem is pure streaming (every x/W element is used exactly once), so
the kernel is HBM-bound. Inputs are cast to fp16 on the host (rel err ~5e-4
vs the 2e-2 gate) which both halves DMA traffic and runs the PE at
1 cycle/row instead of 4 (fp32). Outputs are stored fp16 and upcast on the
host. Per-core traffic: 4 MiB x + 2 MiB W + 1 MiB y = 7 MiB.

Host pre-transposes so every DMA is a plain 2D contiguous slice:
  xh[k, (p*KC + c)*B + b] = x[b, 128c + k, proto]   ([128, 16384] fp16)
  wh[k, (p*KC + c)*E + e] = W[proto, 128c + k, e]   ([128,  8192] fp16)
  y [e, p*B + b]          = out[b, e, proto]        ([128,  4096] fp16)

All of x/W/y fit in SBUF at once (56 KB/partition), so the load path has no
semaphore waits at all: both HWDGE rings (SP + Act) queue 8 chunked loads
each up front and drain at full rate; the tensor engine chases per-chunk
completion sems (one sem per DMA -- HWDGE completions of different DMAs
interleave, so shared counters are racy). Per proto the PE accumulates
out.T = W_p.T @ x_p.T over 4 K-chunks into one of 8 PSUM banks; the DVE
adds bias during the PSUM->SBUF(fp16) copy; stores ride the gpsimd SWDGE
ring except the last protos, which take the (drained) HW rings to shave the
completion-receipt tail. Raw bacc (no Tile) avoids the Tile exit barrier.
"""

import os

import numpy as np

import concourse.bass as bass
from concourse import bacc, mybir
from concourse.bass_utils import run_bass_kernel_spmd

B, D, P, E = 256, 512, 128, 128
NCORES = 8
PL = P // NCORES  # 16 prototypes per core
KC = D // 128  # 4 contraction chunks of 128
GP = 2  # prototypes per DMA chunk
NG = PL // GP  # 8 chunk groups per stream
NPS = 8  # PSUM ring depth (8 banks)

XPW = KC * B  # 1024 x columns per proto
WPW = KC * E  # 512 w columns per proto

_nc_cache = None
LAST_RESULTS = None  # BassKernelResults of the most recent run (for test.py)


def _build_nc() -> bass.Bass:
    nc = bacc.Bacc()
    xh = nc.dram_tensor("xh", [128, PL * XPW], mybir.dt.float16, kind="ExternalInput")
    wh = nc.dram_tensor("wh", [128, PL * WPW], mybir.dt.float16, kind="ExternalInput")
    bT = nc.dram_tensor("bT", [E, PL], mybir.dt.float32, kind="ExternalInput")
    y = nc.dram_tensor("y", [E, PL * B], mybir.dt.float16, kind="ExternalOutput")

    # plain allocs (no context managers): freeing sems/tensors at the end of
    # the program emits a per-semaphore clear storm at kernel exit
    xbuf = nc.alloc_sbuf_tensor("xbuf", [128, PL * XPW], mybir.dt.float16).ap()
    wbuf = nc.alloc_sbuf_tensor("wbuf", [128, PL * WPW], mybir.dt.float16).ap()
    obuf = nc.alloc_sbuf_tensor("obuf", [E, PL * B], mybir.dt.float16).ap()
    btile = nc.alloc_sbuf_tensor("btile", [E, PL], mybir.dt.float32).ap()
    pbuf = [
        nc.alloc_psum_tensor(f"pbuf{i}", [E, B], mybir.dt.float32).ap()
        for i in range(NPS)
    ]

    s_x = [nc.alloc_semaphore(f"s_x{g}") for g in range(NG)]
    s_w = [nc.alloc_semaphore(f"s_w{g}") for g in range(NG)]
    s_mm = nc.alloc_semaphore("s_mm")
    s_vec = nc.alloc_semaphore("s_vec")
    s_b = nc.alloc_semaphore("s_b")
    s_st = nc.alloc_semaphore("s_st")
    s_sthw = nc.alloc_semaphore("s_sthw")

    def xc(g):
        return slice(g * GP * XPW, (g + 1) * GP * XPW)

    def wc(g):
        return slice(g * GP * WPW, (g + 1) * GP * WPW)

    with nc.Block() as block:

        @block.sync
        def _(sync: bass.BassEngine):
            # loads queue up front with no waits; x/w alternate across the
            # two HWDGE rings so each ring carries 3 MiB
            for g in range(NG):
                if g % 2 == 0:
                    sync.dma_start(xbuf[:, xc(g)], xh[:, xc(g)]).then_inc(s_x[g], 16)
                else:
                    sync.dma_start(wbuf[:, wc(g)], wh[:, wc(g)]).then_inc(s_w[g], 16)
            # tail stores on the (drained) HW ring: proto 14, then the first
            # half of proto 15 as soon as the DVE wrote it
            sync.wait_ge(s_vec, 15)
            sync.dma_start(
                y[:, 14 * B : 15 * B], obuf[:, 14 * B : 15 * B]
            ).then_inc(s_sthw, 16)
            sync.wait_ge(s_vec, 16)
            sync.dma_start(
                y[:, 15 * B : 15 * B + B // 2],
                obuf[:, 15 * B : 15 * B + B // 2],
            ).then_inc(s_sthw, 16)
            sync.wait_ge(s_sthw, 48)

        @block.scalar
        def _(scalar: bass.BassEngine):
            for g in range(NG):
                if g % 2 == 0:
                    scalar.dma_start(wbuf[:, wc(g)], wh[:, wc(g)]).then_inc(s_w[g], 16)
                else:
                    scalar.dma_start(xbuf[:, xc(g)], xh[:, xc(g)]).then_inc(s_x[g], 16)
            scalar.wait_ge(s_vec, 17)
            scalar.dma_start(
                y[:, 15 * B + B // 2 : 16 * B],
                obuf[:, 15 * B + B // 2 : 16 * B],
            ).then_inc(s_sthw, 16)
            scalar.wait_ge(s_sthw, 48)

        @block.tensor
        def _(tensor: bass.BassEngine):
            for g in range(NG):
                tensor.wait_ge(s_x[g], 16)
                tensor.wait_ge(s_w[g], 16)
                for p in range(g * GP, (g + 1) * GP):
                    if p >= NPS:
                        tensor.wait_ge(s_vec, p - NPS + 1)
                    for c in range(KC):
                        mm = nc.tensor.matmul(
                            pbuf[p % NPS][:],
                            lhsT=wbuf[:, (p * KC + c) * E : (p * KC + c + 1) * E],
                            rhs=xbuf[:, (p * KC + c) * B : (p * KC + c + 1) * B],
                            start=(c == 0),
                            stop=(c == KC - 1),
                        )
                    mm.then_inc(s_mm, 1)

        @block.vector
        def _(vector: bass.BassEngine):
            vector.wait_ge(s_b, 16)
            for p in range(PL - 1):
                vector.wait_ge(s_mm, p + 1)
                nc.vector.tensor_scalar_add(
                    obuf[:, p * B : (p + 1) * B],
                    pbuf[p % NPS][:],
                    btile[:, p : p + 1],
                ).then_inc(s_vec, 1)
            # last proto in half-B pieces so each half-store launches as soon
            # as its half is written
            p = PL - 1
            vector.wait_ge(s_mm, PL)
            for h in range(2):
                nc.vector.tensor_scalar_add(
                    obuf[:, p * B + h * (B // 2) : p * B + (h + 1) * (B // 2)],
                    pbuf[p % NPS][:, h * (B // 2) : (h + 1) * (B // 2)],
                    btile[:, p : p + 1],
                ).then_inc(s_vec, 1)

        @block.gpsimd
        def _(gpsimd: bass.BassEngine):
            # bias rides the otherwise-idle SWDGE ring
            gpsimd.dma_start(btile[:], bT[:]).then_inc(s_b, 16)
            for a, z in ((0, 4), (4, 8), (8, 12), (12, 14)):
                gpsimd.wait_ge(s_vec, z)
                gpsimd.dma_start(
                    y[:, a * B : z * B], obuf[:, a * B : z * B]
                ).then_inc(s_st, 16)
            gpsimd.wait_ge(s_st, 64)

    nc.compile()
    return nc


def _shard_inputs(x: np.ndarray, W: np.ndarray, b: np.ndarray):
    x16 = x.astype(np.float16)  # [B, D, P]
    W16 = W.astype(np.float16)  # [P, D, E]
    # xr[k, p, c, b] = x[b, 128c + k, p]
    xr = x16.transpose(1, 2, 0).reshape(KC, 128, P, B).transpose(1, 2, 0, 3)
    # wr[k, p, c, e] = W[p, 128c + k, e]
    wr = W16.reshape(P, KC, 128, E).transpose(2, 0, 1, 3)
    bT = np.ascontiguousarray(b.T, dtype=np.float32)  # [E, P]
    in_maps = []
    for m in range(NCORES):
        sl = slice(m * PL, (m + 1) * PL)
        in_maps.append(
            {
                "xh": np.ascontiguousarray(xr[:, sl]).reshape(128, PL * XPW),
                "wh": np.ascontiguousarray(wr[:, sl]).reshape(128, PL * WPW),
                "bT": np.ascontiguousarray(bT[:, sl]),
            }
        )
    return in_maps


def kernel(x: np.ndarray, W: np.ndarray, b: np.ndarray) -> np.ndarray:
    global _nc_cache, LAST_RESULTS
    x = np.asarray(x, dtype=np.float32)
    W = np.asarray(W, dtype=np.float32)
    b = np.asarray(b, dtype=np.float32)
    if _nc_cache is None:
        _nc_cache = _build_nc()
    in_maps = _shard_inputs(x, W, b)
    # one retry: transient device wedges (NRT_EXEC_UNIT_UNRECOVERABLE) have
    # been observed on these shared cores and usually clear on re-execution
    try:
        res = run_bass_kernel_spmd(
            _nc_cache,
            in_maps,
            core_ids=list(range(NCORES)),
            trace=bool(os.environ.get("KERNEL_TRACE")),
        )
    except Exception:
        import time

        time.sleep(5)
        res = run_bass_kernel_spmd(
            _nc_cache,
            in_maps,
            core_ids=list(range(NCORES)),
            trace=False,
        )
    LAST_RESULTS = res
    # y[e, p*B + b] per core -> out[b, e, p]
    outs = [
        r["y"].reshape(E, PL, B).transpose(2, 0, 1) for r in res.results
    ]
    return np.ascontiguousarray(
        np.concatenate(outs, axis=2), dtype=np.float32
    )


# revision 3
# speedup vs baseline: 1.0222x; 1.0222x over previous
"""Trainium2 Bass kernel for nn_Loop_Projection (batched per-prototype GEMM).

Computes out[b, e, p] = sum_d x[b, d, p] * W[p, d, e] + b[p, e] with
x: [256, 512, 128] f32, W: [128, 512, 128] f32, b: [128, 128] f32.

Sharding: prototype axis P=128 split across 8 NeuronCores (16 protos each).
The problem is pure streaming (every x/W element is used exactly once), so
the kernel is HBM-bound. Inputs are cast to fp16 on the host (rel err ~5e-4
vs the 2e-2 gate) which both halves DMA traffic and runs the PE at
1 cycle/row instead of 4 (fp32). Outputs are stored fp16 and upcast on the
host. Per-core traffic: 4 MiB x + 2 MiB W + 1 MiB y = 7 MiB.

Host pre-transposes so every DMA is a plain 2D contiguous slice:
  xh[k, (p*KC + c)*B + b] = x[b, 128c + k, proto]   ([128, 16384] fp16)
  wh[k, (p*KC + c)*E + e] = W[proto, 128c + k, e]   ([128,  8192] fp16)
  y [e, p*B + b]          = out[b, e, proto]        ([128,  4096] fp16)

All of x/W/y fit in SBUF at once (56 KB/partition), so the load path has no
semaphore waits: the two HWDGE rings (SP + Act) queue their chunked loads
up front and drain at full rate, the SWDGE (gpsimd) ring carries one extra
x/w group for queue depth, and the tensor engine chases per-chunk
completion sems (one sem per DMA -- HWDGE completions of different DMAs
interleave, so shared counters are racy). Per proto the PE accumulates
out.T = W_p.T @ x_p.T over 4 K-chunks into one of 8 PSUM banks.

Tail structure (the last ~3µs): the last two chunk groups are single-proto
so the final matmuls finish right behind the last DMA byte; the Act engine
(idle after issuing its loads) evacuates proto 14 from PSUM while the DVE
handles proto 15 in half-B pieces; each HWDGE engine then issues exactly
one store (descriptor generation is ~0.75µs per dma_start, serialized per
engine, so two back-to-back gens on one engine would delay the last store).
Raw bacc (no Tile) avoids the Tile exit barrier.
"""

import os

import numpy as np

import concourse.bass as bass
from concourse import bacc, mybir
from concourse.bass_utils import run_bass_kernel_spmd

B, D, P, E = 256, 512, 128, 128
NCORES = 8
PL = P // NCORES  # 16 prototypes per core
KC = D // 128  # 4 contraction chunks of 128
NPS = 8  # PSUM ring depth (8 banks)

XPW = KC * B  # 1024 x columns per proto
WPW = KC * E  # 512 w columns per proto

# chunk groups: (start proto, end proto). 2-proto groups through proto 13,
# then single-proto groups so the tail matmuls trail the last byte closely.
GROUPS = [(0, 2), (2, 4), (4, 6), (6, 8), (8, 10), (10, 12), (12, 14), (14, 15), (15, 16)]
SWDGE_GROUPS = {6}  # groups whose x+w loads ride the gpsimd SWDGE ring

_nc_cache = None
LAST_RESULTS = None  # BassKernelResults of the most recent run (for test.py)


def _build_nc() -> bass.Bass:
    nc = bacc.Bacc()
    xh = nc.dram_tensor("xh", [128, PL * XPW], mybir.dt.float16, kind="ExternalInput")
    wh = nc.dram_tensor("wh", [128, PL * WPW], mybir.dt.float16, kind="ExternalInput")
    bT = nc.dram_tensor("bT", [E, PL], mybir.dt.float32, kind="ExternalInput")
    y = nc.dram_tensor("y", [E, PL * B], mybir.dt.float16, kind="ExternalOutput")

    # plain allocs (no context managers): freeing sems/tensors at the end of
    # the program emits a per-semaphore clear storm at kernel exit
    xbuf = nc.alloc_sbuf_tensor("xbuf", [128, PL * XPW], mybir.dt.float16).ap()
    wbuf = nc.alloc_sbuf_tensor("wbuf", [128, PL * WPW], mybir.dt.float16).ap()
    obuf = nc.alloc_sbuf_tensor("obuf", [E, PL * B], mybir.dt.float16).ap()
    btile = nc.alloc_sbuf_tensor("btile", [E, PL], mybir.dt.float32).ap()
    pbuf = [
        nc.alloc_psum_tensor(f"pbuf{i}", [E, B], mybir.dt.float32).ap()
        for i in range(NPS)
    ]

    NG = len(GROUPS)
    s_x = [nc.alloc_semaphore(f"s_x{g}") for g in range(NG)]
    s_w = [nc.alloc_semaphore(f"s_w{g}") for g in range(NG)]
    s_mm = nc.alloc_semaphore("s_mm")
    s_vec = nc.alloc_semaphore("s_vec")
    s_act = nc.alloc_semaphore("s_act")
    s_b = nc.alloc_semaphore("s_b")
    s_st = nc.alloc_semaphore("s_st")
    s_sthw = nc.alloc_semaphore("s_sthw")

    def xsl(g):
        a, z = GROUPS[g]
        return slice(a * XPW, z * XPW)

    def wsl(g):
        a, z = GROUPS[g]
        return slice(a * WPW, z * WPW)

    # ring assignment for HWDGE groups: x and w alternate between the two
    # rings per group so both carry ~equal bytes and a group's two halves
    # stream concurrently
    hw_groups = [g for g in range(NG) if g not in SWDGE_GROUPS]

    with nc.Block() as block:

        @block.sync
        def _(sync: bass.BassEngine):
            for i, g in enumerate(hw_groups):
                if i % 2 == 0:
                    sync.dma_start(xbuf[:, xsl(g)], xh[:, xsl(g)]).then_inc(s_x[g], 16)
                else:
                    sync.dma_start(wbuf[:, wsl(g)], wh[:, wsl(g)]).then_inc(s_w[g], 16)
            # one store per HWDGE engine at the tail: protos 12-14 here
            # (12/13 from the DVE stream, 14 from the Act engine)
            sync.wait_ge(s_vec, 14)
            sync.wait_ge(s_act, 1)
            sync.dma_start(
                y[:, 12 * B : 15 * B], obuf[:, 12 * B : 15 * B]
            ).then_inc(s_sthw, 16)
            sync.wait_ge(s_vec, 15)
            sync.dma_start(
                y[:, 15 * B : 15 * B + B // 2],
                obuf[:, 15 * B : 15 * B + B // 2],
            ).then_inc(s_sthw, 16)
            sync.wait_ge(s_sthw, 48)

        @block.scalar
        def _(scalar: bass.BassEngine):
            for i, g in enumerate(hw_groups):
                if i % 2 == 0:
                    scalar.dma_start(wbuf[:, wsl(g)], wh[:, wsl(g)]).then_inc(s_w[g], 16)
                else:
                    scalar.dma_start(xbuf[:, xsl(g)], xh[:, xsl(g)]).then_inc(s_x[g], 16)
            # Act is idle once its loads are queued: evacuate proto 14 from
            # PSUM (copy + bias) so the DVE only owns proto 15 at the tail
            scalar.wait_ge(s_mm, 15)
            scalar.add(
                obuf[:, 14 * B : 15 * B], pbuf[14 % NPS][:], btile[:, 14:15]
            ).then_inc(s_act, 1)
            scalar.wait_ge(s_vec, 16)
            scalar.dma_start(
                y[:, 15 * B + B // 2 : 16 * B],
                obuf[:, 15 * B + B // 2 : 16 * B],
            ).then_inc(s_sthw, 16)
            scalar.wait_ge(s_sthw, 48)

        @block.tensor
        def _(tensor: bass.BassEngine):
            for g in range(len(GROUPS)):
                tensor.wait_ge(s_x[g], 16)
                tensor.wait_ge(s_w[g], 16)
                a, z = GROUPS[g]
                for p in range(a, z):
                    if p >= NPS:
                        tensor.wait_ge(s_vec, p - NPS + 1)
                    for c in range(KC):
                        mm = nc.tensor.matmul(
                            pbuf[p % NPS][:],
                            lhsT=wbuf[:, (p * KC + c) * E : (p * KC + c + 1) * E],
                            rhs=xbuf[:, (p * KC + c) * B : (p * KC + c + 1) * B],
                            start=(c == 0),
                            stop=(c == KC - 1),
                        )
                    mm.then_inc(s_mm, 1)

        @block.vector
        def _(vector: bass.BassEngine):
            vector.wait_ge(s_b, 16)
            for p in range(14):
                vector.wait_ge(s_mm, p + 1)
                nc.vector.tensor_scalar_add(
                    obuf[:, p * B : (p + 1) * B],
                    pbuf[p % NPS][:],
                    btile[:, p : p + 1],
                ).then_inc(s_vec, 1)
            # proto 14 is handled by the Act engine; proto 15 in half-B
            # pieces so each half-store launches as soon as it is written
            p = 15
            vector.wait_ge(s_mm, 16)
            for h in range(2):
                nc.vector.tensor_scalar_add(
                    obuf[:, p * B + h * (B // 2) : p * B + (h + 1) * (B // 2)],
                    pbuf[p % NPS][:, h * (B // 2) : (h + 1) * (B // 2)],
                    btile[:, p : p + 1],
                ).then_inc(s_vec, 1)

        @block.gpsimd
        def _(gpsimd: bass.BassEngine):
            # bias + one x/w group ride the SWDGE ring: a third DMA queue
            # keeps more descriptors in flight against the HBM latency
            gpsimd.dma_start(btile[:], bT[:]).then_inc(s_b, 16)
            for g in sorted(SWDGE_GROUPS):
                gpsimd.dma_start(xbuf[:, xsl(g)], xh[:, xsl(g)]).then_inc(s_x[g], 16)
                gpsimd.dma_start(wbuf[:, wsl(g)], wh[:, wsl(g)]).then_inc(s_w[g], 16)
            for a, z in ((0, 4), (4, 8), (8, 12)):
                gpsimd.wait_ge(s_vec, z)
                gpsimd.dma_start(
                    y[:, a * B : z * B], obuf[:, a * B : z * B]
                ).then_inc(s_st, 16)
            gpsimd.wait_ge(s_st, 48)

    nc.compile()
    return nc


def _shard_inputs(x: np.ndarray, W: np.ndarray, b: np.ndarray):
    x16 = x.astype(np.float16)  # [B, D, P]
    W16 = W.astype(np.float16)  # [P, D, E]
    # xr[k, p, c, b] = x[b, 128c + k, p]
    xr = x16.transpose(1, 2, 0).reshape(KC, 128, P, B).transpose(1, 2, 0, 3)
    # wr[k, p, c, e] = W[p, 128c + k, e]
    wr = W16.reshape(P, KC, 128, E).transpose(2, 0, 1, 3)
    bT = np.ascontiguousarray(b.T, dtype=np.float32)  # [E, P]
    in_maps = []
    for m in range(NCORES):
        sl = slice(m * PL, (m + 1) * PL)
        in_maps.append(
            {
                "xh": np.ascontiguousarray(xr[:, sl]).reshape(128, PL * XPW),
                "wh": np.ascontiguousarray(wr[:, sl]).reshape(128, PL * WPW),
                "bT": np.ascontiguousarray(bT[:, sl]),
            }
        )
    return in_maps


def kernel(x: np.ndarray, W: np.ndarray, b: np.ndarray) -> np.ndarray:
    global _nc_cache, LAST_RESULTS
    x = np.asarray(x, dtype=np.float32)
    W = np.asarray(W, dtype=np.float32)
    b = np.asarray(b, dtype=np.float32)
    if _nc_cache is None:
        _nc_cache = _build_nc()
    in_maps = _shard_inputs(x, W, b)
    # one retry: transient device wedges (NRT_EXEC_UNIT_UNRECOVERABLE) have
    # been observed on these shared cores and usually clear on re-execution
    try:
        res = run_bass_kernel_spmd(
            _nc_cache,
            in_maps,
            core_ids=list(range(NCORES)),
            trace=bool(os.environ.get("KERNEL_TRACE")),
        )
    except Exception:
        import time

        time.sleep(5)
        res = run_bass_kernel_spmd(
            _nc_cache,
            in_maps,
            core_ids=list(range(NCORES)),
            trace=False,
        )
    LAST_RESULTS = res
    # y[e, p*B + b] per core -> out[b, e, p]
    outs = [
        r["y"].reshape(E, PL, B).transpose(2, 0, 1) for r in res.results
    ]
    return np.ascontiguousarray(
        np.concatenate(outs, axis=2), dtype=np.float32
    )
